# revision 5
# baseline (speedup 1.0000x reference)
"""Multi-head causal attention (B=8, T=2048, C=384, H=6, Dh=64) on 8 TRN2 cores.

Sharding: data-parallel over batch — core b computes batch element b end to end
(no collectives).

Per-core kernel layout (all "T" means transposed, head-dim/channel on
partitions):
  xT   [128, 3, 2048]  bf16   c = 128*ci + p
  wq/wk[128, 3, 384]   bf16   packed Wq[h,c,d] -> [c, h*64+d]
  wv   [128, 3, 384]   bf16
  wp   [128, 3, 384]   bf16   Wp[c, e] -> [128, ci, e]
  mask [128, 384]      f32    mask[p, g] = 0 if p <= g-128 else -1e30
  bp   [1, 384]        f32r   bias row (K=1 matmul into output PSUM)

Compute per core:
  QT/KT [hd, t] via matmul(lhsT=w chunk, rhs=xT)      (hd = h*64+d, 3 blocks)
  V_aug [s, 65] per (s-chunk, head), last col = 1     (stationary for PV)
  per q-block j (256 wide), head h:
    ST chunks [s=128, t=256] = KT^T-slice @ QT-slice  (K = d = 64)
    causal mask add on diagonal chunk, exp (ACT, scale=Dh^-0.5) -> P bf16
    O_aug [65, 256] += V_aug^T @ P                    (row 64 = softmax denom)
    recip = 1/denom; B = ones64^T @ recip (K=1)       (broadcast over d)
    attT [hd, t] slice = O[0:64] * B                  (DVE, bf16)
  out [t, e] = attT^T-slice @ wp + ones128^T @ bp     (K = hd, 3 chunks + bias)
"""

import numpy as np
import ml_dtypes

import concourse.bass as bass
import concourse.tile as tile
from concourse import bacc, mybir
from concourse.bass import ts, ds

F32 = mybir.dt.float32
F32R = mybir.dt.float32r
BF16 = mybir.dt.bfloat16
AF = mybir.ActivationFunctionType

B, T, C = 8, 2048, 384
H, DH = 6, 64
SCALE = DH ** -0.5
NEG = -1e30
NCORES = 8
TJ = 256            # q-block width
NJ = T // TJ        # 8 q-blocks
SC = 128            # s-chunk
NCI = C // 128      # 3 channel chunks


def build_kernel():
    nc = bacc.Bacc("TRN2", target_bir_lowering=False, debug=False)

    xT_d = nc.dram_tensor("xT", [128, NCI, T], BF16, kind="ExternalInput").ap()
    wq_d = nc.dram_tensor("wq", [128, NCI, C], BF16, kind="ExternalInput").ap()
    wk_d = nc.dram_tensor("wk", [128, NCI, C], BF16, kind="ExternalInput").ap()
    wv_d = nc.dram_tensor("wv", [128, NCI, C], BF16, kind="ExternalInput").ap()
    wp_d = nc.dram_tensor("wp", [128, NCI, C], BF16, kind="ExternalInput").ap()
    mask_d = nc.dram_tensor("mask", [128, 384], F32, kind="ExternalInput").ap()
    bp_d = nc.dram_tensor("bp", [1, 384], F32R, kind="ExternalInput").ap()
    ones_d = nc.dram_tensor("ones", [1, 128], F32R, kind="ExternalInput").ap()
    y_d = nc.dram_tensor("y", [T, C], F32, kind="ExternalOutput").ap()

    with tile.TileContext(nc) as tc:
        with tc.tile_pool(name="const", bufs=1) as cpool:
            xT = cpool.tile([128, NCI, T], BF16)
            wq = cpool.tile([128, NCI, C], BF16)
            wk = cpool.tile([128, NCI, C], BF16)
            wv = cpool.tile([128, NCI, C], BF16)
            wp = cpool.tile([128, NCI, C], BF16)
            mask = cpool.tile([128, 384], F32)
            bp = cpool.tile([1, 384], F32R)
            ones = cpool.tile([1, 128], F32R)
            QT = cpool.tile([128, NCI, T], BF16)
            KT = cpool.tile([128, NCI, T], BF16)
            attT = cpool.tile([128, NCI, T], BF16)
            Vt = cpool.tile([128, 16, H, 65], BF16)

            for ci in range(NCI):
                nc.sync.dma_start(xT[:, ci, :], xT_d[:, ci, :])
            nc.sync.dma_start(wq[:], wq_d[:])
            nc.sync.dma_start(wk[:], wk_d[:])
            nc.sync.dma_start(wv[:], wv_d[:])
            nc.sync.dma_start(wp[:], wp_d[:])
            nc.sync.dma_start(mask[:], mask_d[:])
            nc.sync.dma_start(bp[:], bp_d[:])
            nc.sync.dma_start(ones[:], ones_d[:])
            # whole-tile memset (contiguous; strided memset fails ISA check);
            # V copies below overwrite cols 0:64, leaving col 64 == 1.0
            nc.gpsimd.memset(Vt[:], 1.0)

            # ---- phase 1: projections ----
            with tc.tile_pool(name="pqk", bufs=2, space="PSUM") as pqk, \
                 tc.tile_pool(name="pv", bufs=2, space="PSUM") as pvp:
                for dst, w in ((QT, wq), (KT, wk)):
                    for pi in range(NCI):
                        for tcn in range(T // 512):
                            ps = pqk.tile([128, 512], F32, tag="pqk")
                            for ci in range(NCI):
                                nc.tensor.matmul(
                                    ps[:],
                                    lhsT=w[:, ci, ts(pi, 128)],
                                    rhs=xT[:, ci, ts(tcn, 512)],
                                    start=(ci == 0), stop=(ci == NCI - 1),
                                )
                            nc.vector.tensor_copy(dst[:, pi, ts(tcn, 512)], ps[:])
                for si in range(16):
                    ps = pvp.tile([128, C], F32, tag="pv")
                    for ci in range(NCI):
                        nc.tensor.matmul(
                            ps[:],
                            lhsT=xT[:, ci, ts(si, 128)],
                            rhs=wv[:, ci, :],
                            start=(ci == 0), stop=(ci == NCI - 1),
                        )
                    nc.vector.tensor_copy(
                        Vt[:, si, :, 0:64],
                        ps[:].rearrange("p (h d) -> p h d", h=H),
                    )

            # ---- phase 2+3: attention + output projection ----
            with tc.tile_pool(name="sps", bufs=3, space="PSUM") as sps, \
                 tc.tile_pool(name="ops", bufs=2, space="PSUM") as ops, \
                 tc.tile_pool(name="bps", bufs=1, space="PSUM") as bps, \
                 tc.tile_pool(name="yps", bufs=1, space="PSUM") as yps, \
                 tc.tile_pool(name="pp", bufs=4) as pp, \
                 tc.tile_pool(name="rp", bufs=2) as rp, \
                 tc.tile_pool(name="yp", bufs=2) as yp:
                for j in range(NJ):
                    for h in range(H):
                        po = (h % 2) * 64     # partition offset inside hd-block
                        bi = h // 2           # hd block index
                        qs = QT[po:po + 64, bi, ts(j, TJ)]
                        O = ops.tile([65, TJ], F32, tag="O")
                        for i in range(2 * j + 1):
                            S = sps.tile([128, TJ], F32, tag="S")
                            nc.tensor.matmul(
                                S[:],
                                lhsT=KT[po:po + 64, bi, ts(i, SC)],
                                rhs=qs,
                                start=True, stop=True,
                            )
                            if i == 2 * j:
                                nc.vector.tensor_add(S[:], S[:], mask[:, 128:384])
                            P = pp.tile([128, TJ], BF16, tag="P")
                            nc.scalar.activation(P[:], S[:], AF.Exp, scale=SCALE)
                            nc.tensor.matmul(
                                O[:],
                                lhsT=Vt[:, i, h, :],
                                rhs=P[:],
                                start=(i == 0), stop=False,
                            )
                        # narrow fringe chunk i = 2j+1 (right half of q-block)
                        i = 2 * j + 1
                        S2 = sps.tile([128, 128], F32, tag="S")
                        nc.tensor.matmul(
                            S2[:],
                            lhsT=KT[po:po + 64, bi, ts(i, SC)],
                            rhs=QT[po:po + 64, bi, ds(j * TJ + 128, 128)],
                            start=True, stop=True,
                        )
                        nc.vector.tensor_add(S2[:], S2[:], mask[:, 128:256])
                        P2 = pp.tile([128, 128], BF16, tag="P")
                        nc.scalar.activation(P2[:], S2[:], AF.Exp, scale=SCALE)
                        nc.tensor.matmul(
                            O[:, 128:256],
                            lhsT=Vt[:, i, h, :],
                            rhs=P2[:],
                            start=False, stop=True,
                        )
                        # normalize: attT[hd, t] = O[d, t] / O[64, t]
                        recip = rp.tile([1, TJ], F32R, tag="recip")
                        with nc.allow_low_precision(reason="f32r recip feeds K=1 broadcast matmul"):
                            nc.vector.reciprocal(recip[:], O[64:65, :])
                        Bb = bps.tile([64, TJ], F32, tag="B")
                        nc.tensor.matmul(Bb[:], lhsT=ones[:, 0:64], rhs=recip[:],
                                         start=True, stop=True)
                        # DVE can read only one PSUM operand: stage B in SBUF
                        Bsb = rp.tile([64, TJ], F32, tag="Bsb")
                        nc.vector.tensor_copy(Bsb[:], Bb[:])
                        nc.vector.tensor_mul(
                            attT[po:po + 64, bi, ts(j, TJ)], O[0:64, :], Bsb[:]
                        )
                    # ---- output projection for the two 128-t-blocks of j ----
                    for tb in (2 * j, 2 * j + 1):
                        Y = yps.tile([128, C], F32, tag="Y")
                        for ci in range(NCI):
                            nc.tensor.matmul(
                                Y[:],
                                lhsT=attT[:, ci, ts(tb, 128)],
                                rhs=wp[:, ci, :],
                                start=(ci == 0), stop=False,
                            )
                        nc.tensor.matmul(Y[:], lhsT=ones[:], rhs=bp[:],
                                         start=False, stop=True)
                        ysb = yp.tile([128, C], F32, tag="ysb")
                        nc.vector.tensor_copy(ysb[:], Y[:])
                        nc.sync.dma_start(y_d[ts(tb, 128), :], ysb[:])

    nc.compile()
    return nc


def _prep_inputs(x, Wq, Wk, Wv, Wp, bp):
    """Host-side shard + layout prep. Returns per-core input maps."""
    bf = ml_dtypes.bfloat16
    x = np.asarray(x, dtype=np.float32)

    def pack_w(W):  # [H, C, Dh] -> [128, NCI, H*Dh]
        Whd = np.transpose(np.asarray(W, np.float32), (1, 0, 2)).reshape(C, H * DH)
        return np.ascontiguousarray(
            Whd.reshape(NCI, 128, H * DH).transpose(1, 0, 2)
        ).astype(bf)

    wq_p, wk_p, wv_p = pack_w(Wq), pack_w(Wk), pack_w(Wv)
    wp_p = np.ascontiguousarray(
        np.asarray(Wp, np.float32).reshape(NCI, 128, C).transpose(1, 0, 2)
    ).astype(bf)

    g = np.arange(384)[None, :]
    p = np.arange(128)[:, None]
    mask = np.where(p <= g - 128, 0.0, NEG).astype(np.float32)
    bp_row = np.asarray(bp, np.float32).reshape(1, C)
    ones_row = np.ones((1, 128), np.float32)

    in_maps = []
    for b in range(B):
        xT = np.ascontiguousarray(
            x[b].T.reshape(NCI, 128, T).transpose(1, 0, 2)
        ).astype(bf)
        in_maps.append({
            "xT": xT, "wq": wq_p, "wk": wk_p, "wv": wv_p, "wp": wp_p,
            "mask": mask, "bp": bp_row, "ones": ones_row,
        })
    return in_maps


_CACHE = {}


def kernel(x, Wq, Wk, Wv, Wp, bp):
    from concourse.bass_utils import run_bass_kernel_spmd

    if "nc" not in _CACHE:
        _CACHE["nc"] = build_kernel()
    nc = _CACHE["nc"]
    in_maps = _prep_inputs(x, Wq, Wk, Wv, Wp, bp)
    res = run_bass_kernel_spmd(nc, in_maps, list(range(NCORES)))
    out = np.stack([res.results[b]["y"] for b in range(B)], axis=0)
    return out.astype(np.float32)


# revision 7
# speedup vs baseline: 1.1079x; 1.1079x over previous
"""Multi-head causal attention (B=8, T=2048, C=384, H=6, Dh=64) on 8 TRN2 cores.

Sharding: data-parallel over batch — core b computes batch element b end to end
(no collectives).

Per-core kernel layout (all "T" means transposed, head-dim/channel on
partitions):
  xT   [128, 3, 2048]  bf16   c = 128*ci + p
  wq/wk[128, 3, 384]   bf16   packed Wq[h,c,d] -> [c, h*64+d]
  wv   [128, 3, 384]   bf16
  wp   [128, 3, 384]   bf16   Wp[c, e] -> [128, ci, e]
  mask [128, 384]      f32    mask[p, g] = 0 if p <= g-128 else -1e30
  bp   [1, 384]        f32r   bias row (K=1 matmul into output PSUM)

Compute per core:
  QT/KT [hd, t] via matmul(lhsT=w chunk, rhs=xT)      (hd = h*64+d, 3 blocks)
  V_aug [s, 65] per (s-chunk, head), last col = 1     (stationary for PV)
  per q-block j (256 wide), head h:
    ST chunks [s=128, t=256] = KT^T-slice @ QT-slice  (K = d = 64)
    causal mask add on diagonal chunk, exp (ACT, scale=Dh^-0.5) -> P bf16
    O_aug [65, 256] += V_aug^T @ P                    (row 64 = softmax denom)
    recip = 1/denom; B = ones64^T @ recip (K=1)       (broadcast over d)
    attT [hd, t] slice = O[0:64] * B                  (DVE, bf16)
  out [t, e] = attT^T-slice @ wp + ones128^T @ bp     (K = hd, 3 chunks + bias)
"""

import numpy as np
import ml_dtypes

import concourse.bass as bass
import concourse.tile as tile
from concourse import bacc, mybir
from concourse.bass import ts, ds

F32 = mybir.dt.float32
F32R = mybir.dt.float32r
BF16 = mybir.dt.bfloat16
AF = mybir.ActivationFunctionType

B, T, C = 8, 2048, 384
H, DH = 6, 64
SCALE = DH ** -0.5
NEG = -1e30
NCORES = 8
TJ = 512            # q-block width
NJ = T // TJ        # 8 q-blocks
SC = 128            # s-chunk
NCI = C // 128      # 3 channel chunks


def build_kernel():
    nc = bacc.Bacc("TRN2", target_bir_lowering=False, debug=False)

    xT_d = nc.dram_tensor("xT", [128, NCI, T], BF16, kind="ExternalInput").ap()
    wq_d = nc.dram_tensor("wq", [128, NCI, C], BF16, kind="ExternalInput").ap()
    wk_d = nc.dram_tensor("wk", [128, NCI, C], BF16, kind="ExternalInput").ap()
    wv_d = nc.dram_tensor("wv", [128, NCI, C], BF16, kind="ExternalInput").ap()
    wp_d = nc.dram_tensor("wp", [128, NCI, C], BF16, kind="ExternalInput").ap()
    mask_d = nc.dram_tensor("mask", [128, 128], F32, kind="ExternalInput").ap()
    bp_d = nc.dram_tensor("bp", [1, 384], F32R, kind="ExternalInput").ap()
    ones_d = nc.dram_tensor("ones", [1, 128], F32R, kind="ExternalInput").ap()
    y_d = nc.dram_tensor("y", [T, C], F32, kind="ExternalOutput").ap()

    with tile.TileContext(nc) as tc:
        with tc.tile_pool(name="const", bufs=1) as cpool:
            xT = cpool.tile([128, NCI, T], BF16)
            wq = cpool.tile([128, NCI, C], BF16)
            wk = cpool.tile([128, NCI, C], BF16)
            wv = cpool.tile([128, NCI, C], BF16)
            wp = cpool.tile([128, NCI, C], BF16)
            mask = cpool.tile([128, 128], F32)
            bp = cpool.tile([1, 384], F32R)
            ones = cpool.tile([1, 128], F32R)
            QT = cpool.tile([128, NCI, T], BF16)
            KT = cpool.tile([128, NCI, T], BF16)
            attT = cpool.tile([128, NCI, T], BF16)
            Vt = cpool.tile([128, 16, H, 65], BF16)

            for ci in range(NCI):
                nc.sync.dma_start(xT[:, ci, :], xT_d[:, ci, :])
            nc.sync.dma_start(wq[:], wq_d[:])
            nc.sync.dma_start(wk[:], wk_d[:])
            nc.sync.dma_start(wv[:], wv_d[:])
            nc.sync.dma_start(wp[:], wp_d[:])
            nc.sync.dma_start(mask[:], mask_d[:])
            nc.sync.dma_start(bp[:], bp_d[:])
            nc.sync.dma_start(ones[:], ones_d[:])
            # whole-tile memset (contiguous; strided memset fails ISA check);
            # V copies below overwrite cols 0:64, leaving col 64 == 1.0
            nc.gpsimd.memset(Vt[:], 1.0)

            # ---- phase 1: projections ----
            with tc.tile_pool(name="pqk", bufs=2, space="PSUM") as pqk, \
                 tc.tile_pool(name="pv", bufs=2, space="PSUM") as pvp:
                for dst, w in ((QT, wq), (KT, wk)):
                    for pi in range(NCI):
                        for tcn in range(T // 512):
                            ps = pqk.tile([128, 512], F32, tag="pqk")
                            for ci in range(NCI):
                                nc.tensor.matmul(
                                    ps[:],
                                    lhsT=w[:, ci, ts(pi, 128)],
                                    rhs=xT[:, ci, ts(tcn, 512)],
                                    start=(ci == 0), stop=(ci == NCI - 1),
                                )
                            nc.vector.tensor_copy(dst[:, pi, ts(tcn, 512)], ps[:])
                for si in range(16):
                    ps = pvp.tile([128, C], F32, tag="pv")
                    for ci in range(NCI):
                        nc.tensor.matmul(
                            ps[:],
                            lhsT=xT[:, ci, ts(si, 128)],
                            rhs=wv[:, ci, :],
                            start=(ci == 0), stop=(ci == NCI - 1),
                        )
                    nc.vector.tensor_copy(
                        Vt[:, si, :, 0:64],
                        ps[:].rearrange("p (h d) -> p h d", h=H),
                    )

            # ---- phase 2+3: attention + output projection ----
            with tc.tile_pool(name="sps", bufs=3, space="PSUM") as sps, \
                 tc.tile_pool(name="ops", bufs=2, space="PSUM") as ops, \
                 tc.tile_pool(name="bps", bufs=1, space="PSUM") as bps, \
                 tc.tile_pool(name="yps", bufs=1, space="PSUM") as yps, \
                 tc.tile_pool(name="pp", bufs=4) as pp, \
                 tc.tile_pool(name="rp", bufs=2) as rp, \
                 tc.tile_pool(name="yp", bufs=2) as yp:
                NCH = TJ // SC  # s-chunks per q-block (4)
                for j in range(NJ):
                    for h in range(H):
                        po = (h % 2) * 64     # partition offset inside hd-block
                        bi = h // 2           # hd block index
                        O = ops.tile([65, TJ], F32, tag="O")
                        for i in range(NCH * j + NCH):
                            fringe = i >= NCH * j
                            d = SC * i - TJ * j if fringe else 0
                            S = sps.tile([128, TJ], F32, tag="S")
                            nc.tensor.matmul(
                                S[:, d:TJ],
                                lhsT=KT[po:po + 64, bi, ts(i, SC)],
                                rhs=QT[po:po + 64, bi, ds(j * TJ + d, TJ - d)],
                                start=True, stop=True,
                            )
                            if fringe:
                                # only cols [d, d+128) straddle the diagonal
                                nc.vector.tensor_add(
                                    S[:, d:d + 128], S[:, d:d + 128], mask[:]
                                )
                            P = pp.tile([128, TJ], BF16, tag="P")
                            if d > 0:
                                nc.gpsimd.memset(P[:, 0:d], 0.0)
                            nc.scalar.activation(P[:, d:TJ], S[:, d:TJ],
                                                 AF.Exp, scale=SCALE)
                            nc.tensor.matmul(
                                O[:],
                                lhsT=Vt[:, i, h, :],
                                rhs=P[:],
                                start=(i == 0), stop=(i == NCH * j + NCH - 1),
                            )
                        # normalize: attT[hd, t] = O[d, t] / O[64, t]
                        recip = rp.tile([1, TJ], F32R, tag="recip")
                        with nc.allow_low_precision(reason="f32r recip feeds K=1 broadcast"):
                            nc.vector.reciprocal(recip[:], O[64:65, :])
                        Bb = bps.tile([64, TJ], F32, tag="B")
                        nc.tensor.matmul(Bb[:], lhsT=ones[:, 0:64], rhs=recip[:],
                                         start=True, stop=True)
                        # DVE can read only one PSUM operand: stage B in SBUF
                        Bsb = rp.tile([64, TJ], F32, tag="Bsb")
                        nc.vector.tensor_copy(Bsb[:], Bb[:])
                        nc.vector.tensor_mul(
                            attT[po:po + 64, bi, ts(j, TJ)], O[0:64, :], Bsb[:]
                        )
                    # ---- output projection for the 128-t-blocks of j ----
                    for tb in range(4 * j, 4 * j + 4):
                        Y = yps.tile([128, C], F32, tag="Y")
                        for ci in range(NCI):
                            nc.tensor.matmul(
                                Y[:],
                                lhsT=attT[:, ci, ts(tb, 128)],
                                rhs=wp[:, ci, :],
                                start=(ci == 0), stop=False,
                            )
                        nc.tensor.matmul(Y[:], lhsT=ones[:], rhs=bp[:],
                                         start=False, stop=True)
                        ysb = yp.tile([128, C], F32, tag="ysb")
                        nc.vector.tensor_copy(ysb[:], Y[:])
                        nc.sync.dma_start(y_d[ts(tb, 128), :], ysb[:])

    nc.compile()
    return nc


def _prep_inputs(x, Wq, Wk, Wv, Wp, bp):
    """Host-side shard + layout prep. Returns per-core input maps."""
    bf = ml_dtypes.bfloat16
    x = np.asarray(x, dtype=np.float32)

    def pack_w(W):  # [H, C, Dh] -> [128, NCI, H*Dh]
        Whd = np.transpose(np.asarray(W, np.float32), (1, 0, 2)).reshape(C, H * DH)
        return np.ascontiguousarray(
            Whd.reshape(NCI, 128, H * DH).transpose(1, 0, 2)
        ).astype(bf)

    wq_p, wk_p, wv_p = pack_w(Wq), pack_w(Wk), pack_w(Wv)
    wp_p = np.ascontiguousarray(
        np.asarray(Wp, np.float32).reshape(NCI, 128, C).transpose(1, 0, 2)
    ).astype(bf)

    f = np.arange(128)[None, :]
    p = np.arange(128)[:, None]
    mask = np.where(p <= f, 0.0, NEG).astype(np.float32)
    bp_row = np.asarray(bp, np.float32).reshape(1, C)
    ones_row = np.ones((1, 128), np.float32)

    in_maps = []
    for b in range(B):
        xT = np.ascontiguousarray(
            x[b].T.reshape(NCI, 128, T).transpose(1, 0, 2)
        ).astype(bf)
        in_maps.append({
            "xT": xT, "wq": wq_p, "wk": wk_p, "wv": wv_p, "wp": wp_p,
            "mask": mask, "bp": bp_row, "ones": ones_row,
        })
    return in_maps


_CACHE = {}


def kernel(x, Wq, Wk, Wv, Wp, bp):
    from concourse.bass_utils import run_bass_kernel_spmd

    if "nc" not in _CACHE:
        _CACHE["nc"] = build_kernel()
    nc = _CACHE["nc"]
    in_maps = _prep_inputs(x, Wq, Wk, Wv, Wp, bp)
    res = run_bass_kernel_spmd(nc, in_maps, list(range(NCORES)))
    out = np.stack([res.results[b]["y"] for b in range(B)], axis=0)
    return out.astype(np.float32)
